# revision 26
# baseline (speedup 1.0000x reference)
"""Bass/Tile TRN2 kernel for EnhancedIPA3 (invariant-point-attention variant).

v3 strategy: 8 cores = batch(2) x query-block(4), **no collectives**.
Each core receives the FULL (host-transposed, bf16) s for its batch and
computes K-side features for all 1024 keys locally; only Q-side work and
attention are sharded by query block (key rows are host-permuted so each
core's own block sits at tiles 0-1, letting one SPMD program serve all
cores).  Feature transposes ride the DMA xbar (SBUF->DRAM->SBUF
dma_start_transpose).  Per-head softmax temperature (sigmoid(head_w)) is
folded into the EXP activation scale; q-feature scales are folded into
wq / the gates, eliminating the qscale multiply.

Self-contained: hardcodes all shapes; only depends on numpy + concourse.
"""

import numpy as np
import ml_dtypes
from contextlib import ExitStack

import concourse.bass as bass
import concourse.bacc as bacc
import concourse.mybir as mybir
import concourse.tile as tile
from concourse.bass_utils import run_bass_kernel_spmd
from concourse.masks import make_identity

F32 = mybir.dt.float32
F32R = mybir.dt.float32r
BF16 = mybir.dt.bfloat16
AF = mybir.ActivationFunctionType
OP = mybir.AluOpType

B, N, CS, H, C, P, V = 2, 1024, 384, 12, 16, 4, 8
EPS = 1e-8
NB = N // 4              # 256 query rows per core
NT = NB // 128           # 2 query row-tiles per core
RT = N // 128            # 8 key row-tiles (all computed locally)
KVP = P + V              # 12 kv points per head
FEAT = 64                # padded per-head attention feature stride
FS = 42                  # used attention features per head
FOUT = H * (C + 7 * V)   # 864 output-proj input channels
FPAD = 896               # feats padded to 7*128 (ones at 864:866, zero pad)
KCH = 7                  # contraction chunks for output proj
VLD = 72                 # per-head va block: vs 16 | 6 comps x 8 | ones | pad7
VCOLS = H * VLD          # 864 va columns

# wallK columns: per comp j: [kp (h,4)=48 | vp (h,8)=96]; then k, v scalars
WKP = 6 * H * KVP        # 864
WK_COLS = WKP + 192 + 192            # 1248
# wallQ columns: [q-pts comp-major 6*48 | q (h,c) 192 | g (h,4) 48]
WQP = 6 * H * P          # 288
WQ_COLS = WQP + 192 + 48             # 528


def _host_prep(inputs):
    """Layout-only host prep: transposes, dtype casts, col permutations,
    and folding of per-head scales into weights/gates/exp-scale."""
    wq = np.asarray(inputs["wq"], np.float32)
    wkv = np.asarray(inputs["wkv"], np.float32)
    wqp = np.asarray(inputs["wqp"], np.float32)
    wkvp = np.asarray(inputs["wkvp"], np.float32)
    wg = np.asarray(inputs["wg"], np.float32)
    biases = [np.asarray(inputs[k], np.float32)
              for k in ("bq", "bkv", "bqp", "bkvp", "bg")]
    has_bias = any(np.abs(b).max() > 0 for b in biases)
    bq, bkv, bqp, bkvp, bg = biases
    gw = np.asarray(inputs["geom_weight"], np.float32)
    hw = np.asarray(inputs["head_weights"], np.float32)
    sh = 1.0 / (1.0 + np.exp(-hw))           # sigmoid(head_weights) [H]

    # ---- wallK [384+1, 1248] ----
    wallK = np.zeros((CS + 1, WK_COLS), np.float32)
    wkvp_r = wkvp.reshape(CS, H, KVP, 6)
    bkvp_r = bkvp.reshape(H, KVP, 6)
    for j in range(6):
        o = j * 144
        wallK[:CS, o:o + 48] = wkvp_r[:, :, :P, j].reshape(CS, 48)
        wallK[CS, o:o + 48] = bkvp_r[:, :P, j].reshape(-1)
        wallK[:CS, o + 48:o + 144] = wkvp_r[:, :, P:, j].reshape(CS, 96)
        wallK[CS, o + 48:o + 144] = bkvp_r[:, P:, j].reshape(-1)
    wallK[:CS, WKP:WKP + 192] = wkv[:, :192]
    wallK[CS, WKP:WKP + 192] = bkv[:192]
    wallK[:CS, WKP + 192:] = wkv[:, 192:]
    wallK[CS, WKP + 192:] = bkv[192:]

    # ---- wallQ [384+1, 528]; wq pre-scaled by 1/sqrt(C) ----
    wallQ = np.zeros((CS + 1, WQ_COLS), np.float32)
    wqp_r = wqp.reshape(CS, H, P, 6)
    bqp_r = bqp.reshape(H, P, 6)
    for j in range(6):
        wallQ[:CS, j * H * P:(j + 1) * H * P] = \
            wqp_r[:, :, :, j].reshape(CS, H * P)
        wallQ[CS, j * H * P:(j + 1) * H * P] = bqp_r[:, :, j].reshape(-1)
    cs_scale = 1.0 / np.sqrt(C)
    wallQ[:CS, WQP:WQP + 192] = wq * cs_scale
    wallQ[CS, WQP:WQP + 192] = bq * cs_scale
    wallQ[:CS, WQP + 192:] = wg
    wallQ[CS, WQP + 192:] = bg

    def pack_chunks(w):
        cols = w.shape[1]
        out = np.zeros((128, 3, cols), np.float32)
        for kc in range(3):
            out[:, kc, :] = w[kc * 128:(kc + 1) * 128, :]
        return out.reshape(128, -1).astype(ml_dtypes.bfloat16)

    wallK_p = pack_chunks(wallK[:CS])
    wallQ_p = pack_chunks(wallQ[:CS])
    wbias = np.concatenate([wallK[CS:], wallQ[CS:]],
                           axis=1).astype(ml_dtypes.bfloat16)  # [1, 1776]

    # ---- wout packed [128, 7, 384] ----
    wout = np.asarray(inputs["wout"], np.float32)
    bout_half = np.asarray(inputs["bout"], np.float32)[None, :] * 0.5
    wout_b = np.concatenate([wout, bout_half, bout_half], axis=0)  # [866,384]
    woutp = np.zeros((128, KCH, CS), np.float32)
    for kc in range(KCH):
        r0 = kc * 128
        r1 = min(866, r0 + 128)
        woutp[:r1 - r0, kc, :] = wout_b[r0:r1]
    woutp = woutp.reshape(128, -1).astype(ml_dtypes.bfloat16)

    # gate scales & combo-column coefficients (qscale elimination)
    gsc_co = gw[0] * 0.5 if gw[0] != 0 else 1.0   # coord-slot gate scale
    gsc_di = gw[1] if gw[1] != 0 else 1.0          # dir-slot gate scale
    c_q2 = (-gw[0] / P) / (gsc_co * gsc_co) if gw[0] != 0 else 0.0
    c_curv = -gw[1] / P

    # ---- per-core: sT packed + rot/trans (rows permuted, own block first)
    s = np.asarray(inputs["s"], np.float32)
    rot = np.asarray(inputs["rot"], np.float32).reshape(B, N, 9)
    trans = np.asarray(inputs["trans"], np.float32)
    sT_p, rt12_p = [], []
    for c in range(8):
        b, qb = c // 4, c % 4
        perm = [qb] + [x for x in range(4) if x != qb]
        ridx = np.concatenate([np.arange(p * NB, (p + 1) * NB) for p in perm])
        sT = np.ascontiguousarray(s[b][ridx].T)    # [384, 1024]
        stp = sT.reshape(3, 128, N).transpose(1, 0, 2).reshape(128, 3 * N)
        sT_p.append(stp.astype(ml_dtypes.bfloat16))
        rt = np.concatenate([rot[b][ridx], trans[b][ridx]], axis=1)
        rtp = rt.reshape(RT, 128, 12).transpose(1, 0, 2).reshape(128, RT * 12)
        rt12_p.append(rtp.astype(np.float32))

    return dict(wallK=wallK_p, wallQ=wallQ_p, wbias=wbias, woutp=woutp,
                sT=sT_p, rt12=rt12_p, gw=gw, sh=sh, has_bias=has_bias,
                gsc_co=gsc_co, gsc_di=gsc_di, c_q2=c_q2, c_curv=c_curv)


_PROGRAM_CACHE = {}


def _build_program(hp):
    key = (tuple(np.round(hp["sh"], 7).tolist()), float(hp["gw"][0]),
           float(hp["gw"][1]), bool(hp["has_bias"]))
    if key in _PROGRAM_CACHE:
        return _PROGRAM_CACHE[key]

    nc = bacc.Bacc("TRN2", target_bir_lowering=False, debug=False,
                   num_devices=8)

    sT_d = nc.dram_tensor("sT", [128, 3 * N], BF16, kind="ExternalInput")
    rt12_d = nc.dram_tensor("rt12", [128, RT * 12], F32, kind="ExternalInput")
    wallK_d = nc.dram_tensor("wallK", [128, 3 * WK_COLS], BF16,
                             kind="ExternalInput")
    wallQ_d = nc.dram_tensor("wallQ", [128, 3 * WQ_COLS], BF16,
                             kind="ExternalInput")
    woutp_d = nc.dram_tensor("woutp", [128, KCH * CS], BF16,
                             kind="ExternalInput")
    wbias_d = nc.dram_tensor("wbias", [1, WK_COLS + WQ_COLS], BF16,
                             kind="ExternalInput") if hp["has_bias"] else None
    out_d = nc.dram_tensor("out_loc", [NB, CS], F32, kind="ExternalOutput")

    with tile.TileContext(nc) as tc:
        with ExitStack() as ctx:
            _emit(ctx, tc, nc, sT_d, rt12_d, wallK_d, wallQ_d, wbias_d,
                  woutp_d, out_d, hp)

    nc.compile()
    _PROGRAM_CACHE[key] = nc
    return nc


def _emit(ctx, tc, nc, sT_d, rt12_d, wallK_d, wallQ_d, wbias_d, woutp_d,
          out_d, hp):
    PS = bass.MemorySpace.PSUM
    gw0, gw1 = float(hp["gw"][0]), float(hp["gw"][1])
    sh = hp["sh"]
    has_bias = hp["has_bias"]

    const = ctx.enter_context(tc.tile_pool(name="const", bufs=1))
    work = ctx.enter_context(tc.tile_pool(name="work", bufs=1))
    tmp = ctx.enter_context(tc.tile_pool(name="tmp", bufs=2))

    # ---- constant loads ---------------------------------------------------
    sT_sb = const.tile([128, 3 * N], BF16, name="sT")
    wallK = const.tile([128, 3 * WK_COLS], BF16, name="wallK")
    for kc in range(3):
        nc.sync.dma_start(sT_sb[:, kc * N:(kc + 1) * N],
                          sT_d[:, kc * N:(kc + 1) * N])
        nc.sync.dma_start(wallK[:, kc * WK_COLS:(kc + 1) * WK_COLS],
                          wallK_d[:, kc * WK_COLS:(kc + 1) * WK_COLS])
    rt12 = const.tile([128, RT * 12], F32, name="rt12")
    nc.sync.dma_start(rt12[:], rt12_d[:, :])
    wallQ = const.tile([128, 3 * WQ_COLS], BF16, name="wallQ")
    nc.sync.dma_start(wallQ[:], wallQ_d[:, :])
    woutp = const.tile([128, KCH * CS], BF16, name="woutp")
    nc.sync.dma_start(woutp[:], woutp_d[:, :])
    if has_bias:
        wbias = const.tile([1, WK_COLS + WQ_COLS], BF16, name="wbias")
        nc.sync.dma_start(wbias[:], wbias_d[:, :])
        ones1 = const.tile([1, N], BF16, name="ones1")
        nc.gpsimd.memset(ones1[:], 1.0)

    ident = const.tile([128, 128], F32, name="ident")
    make_identity(nc, ident[:])
    ident_r = const.tile([VLD, VLD], F32R, name="identr")
    nc.vector.tensor_copy(ident_r[:], ident[0:VLD, 0:VLD])
    identb = const.tile([128, 128], BF16, name="identb")
    nc.vector.tensor_copy(identb[:], ident[:])
    # pin the sigmoid table set before any relu evacuations
    actpin = const.tile([1, 1], F32, name="actpin")
    nc.scalar.activation(actpin[:], ident[0:1, 0:1], AF.Sigmoid)

    # ---- persistent feature tiles ----------------------------------------
    kf = [work.tile([128, FEAT * H], BF16, name=f"kf{rt}") for rt in range(RT)]
    va = [work.tile([128, VCOLS], BF16, name=f"va{rt}") for rt in range(RT)]
    qf = [work.tile([128, FEAT * H], BF16, name=f"qf{nt}") for nt in range(NT)]
    gc_sb = [work.tile([128, H * P], BF16, name=f"gc{nt}") for nt in range(NT)]
    gd_sb = [work.tile([128, H * P], BF16, name=f"gd{nt}") for nt in range(NT)]

    kfT = [work.tile([128, N], BF16, name=f"kfT{t}") for t in range(6)]
    qfT = [work.tile([128, NB], BF16, name=f"qfT{t}") for t in range(6)]
    kfv = [t[:].rearrange("p (h f) -> p h f", f=FEAT) for t in kf]
    vav = [t[:].rearrange("p (h f) -> p h f", f=VLD) for t in va]
    qfv = [t[:].rearrange("p (h f) -> p h f", f=FEAT) for t in qf]

    ppool = ExitStack()
    ppsum = ppool.enter_context(tc.tile_pool(name="ppsum", bufs=3, space=PS))
    tpsum = ppool.enter_context(tc.tile_pool(name="tpsum", bufs=2, space=PS))
    KC = 3

    def pe_transpose(dst, src_ap, t_idx):
        ps = tpsum.tile([128, 128], BF16, tag="tps", name="tps")
        nc.tensor.transpose(ps[:], src_ap, identb[:])
        if t_idx % 2:
            nc.scalar.copy(dst, ps[:])
        else:
            nc.vector.tensor_copy(dst, ps[:])

    def proj(psv, wall_sb, wcols, c0, c1, colbase, bias_off):
        for kc in range(KC):
            nc.tensor.matmul(
                psv,
                sT_sb[:, kc * N + colbase:kc * N + colbase + 128],
                wall_sb[:, kc * wcols + c0:kc * wcols + c1],
                start=(kc == 0), stop=(kc == KC - 1 and not has_bias))
        if has_bias:
            nc.tensor.matmul(psv, ones1[:, colbase:colbase + 128],
                             wbias[:, bias_off + c0:bias_off + c1],
                             start=False, stop=True)

    def emit_ktile(rt):
        colbase = rt * 128
        rot = rt12[:, rt * 12:rt * 12 + 9]
        tr = rt12[:, rt * 12 + 9:rt * 12 + 12]
        W = H * KVP  # 144
        fma_eng = nc.vector

        ps_co = ppsum.tile([128, 3 * W], F32, tag="proj", name="psco")
        proj(ps_co[:], wallK, WK_COLS, 0, 3 * W, colbase, 0)
        ps_di = ppsum.tile([128, 3 * W], F32, tag="proj", name="psdi")
        proj(ps_di[:], wallK, WK_COLS, 3 * W, WKP, colbase, 0)
        ps_kv = ppsum.tile([128, 384], F32, tag="proj", name="pskv")
        proj(ps_kv[:], wallK, WK_COLS, WKP, WK_COLS, colbase, 0)

        # evacuate: relu pts on DVE; k scalars ACT; v scalars ACT (va 0:192)
        pts = tmp.tile([128, WKP], BF16, tag="pts", name="pts", bufs=3)
        nc.scalar.activation(pts[:, 0:3 * W], ps_co[:], AF.Relu)
        nc.vector.tensor_scalar_max(pts[:, 3 * W:6 * W], ps_di[:], 0.0)
        nc.scalar.copy(
            kfv[rt][:, :, 0:16],
            ps_kv[:, 0:192].rearrange("p (h c) -> p h c", c=16))
        nc.scalar.copy(
            vav[rt][:, :, 0:16],
            ps_kv[:, 192:384].rearrange("p (h c) -> p h c", c=16))

        # rigid transform into pco: init on gpsimd, fma on DVE -------------
        pco = tmp.tile([128, WKP], BF16, tag="pco", name="pco", bufs=3)
        for i in range(3):
            dco = pco[:, i * W:(i + 1) * W]
            nc.scalar.activation(dco, pts[:, 0:W], AF.Identity,
                                 bias=tr[:, i:i + 1],
                                 scale=rot[:, 3 * i:3 * i + 1])
            fma_eng.scalar_tensor_tensor(dco, pts[:, W:2 * W],
                                         rot[:, 3 * i + 1:3 * i + 2], dco,
                                         OP.mult, OP.add)
            fma_eng.scalar_tensor_tensor(dco, pts[:, 2 * W:3 * W],
                                         rot[:, 3 * i + 2:3 * i + 3], dco,
                                         OP.mult, OP.add)
            ddi = pco[:, (3 + i) * W:(4 + i) * W]
            nc.scalar.activation(ddi, pts[:, 3 * W:4 * W], AF.Identity,
                                 scale=rot[:, 3 * i:3 * i + 1])
            fma_eng.scalar_tensor_tensor(ddi, pts[:, 4 * W:5 * W],
                                         rot[:, 3 * i + 1:3 * i + 2], ddi,
                                         OP.mult, OP.add)
            fma_eng.scalar_tensor_tensor(ddi, pts[:, 5 * W:6 * W],
                                         rot[:, 3 * i + 2:3 * i + 3], ddi,
                                         OP.mult, OP.add)

        # kp -> kf slots (ACT, strided); vp -> va groups (gpsimd, contig) --
        for j in range(6):
            kp_dst = kfv[rt][:, :, 16 + 4 * j:20 + 4 * j]
            kp_src = pco[:, j * W:j * W + 48].rearrange("p (h x) -> p h x", x=P)
            if j % 2:
                nc.scalar.copy(kp_dst, kp_src)
            else:
                nc.vector.tensor_copy(kp_dst, kp_src)
            nc.vector.tensor_copy(
                vav[rt][:, :, 16 + 8 * j:24 + 8 * j],
                pco[:, j * W + 48:(j + 1) * W].rearrange(
                    "p (h x) -> p h x", x=V))

        # k2 term -> kf col 40 ----------------------------------------------
        sq = tmp.tile([128, H * 12], F32, tag="sq", name="sq")
        cslots = kfv[rt][:, :, 16:28]
        nc.vector.tensor_tensor(sq[:].rearrange("p (h x) -> p h x", x=12),
                                cslots, cslots, OP.mult)
        k2 = tmp.tile([128, H], F32, tag="k2", name="k2")
        nc.vector.tensor_reduce(k2[:],
                                sq[:].rearrange("p (h x) -> p h x", x=12),
                                mybir.AxisListType.X, OP.add)
        nc.vector.tensor_scalar_mul(kfv[rt][:, :, 40], k2[:], -gw0 / P)
        nc.gpsimd.memset(kfv[rt][:, :, 41], 1.0)
        nc.gpsimd.memset(kfv[rt][:, :, 42:64], 0.0)
        nc.gpsimd.memset(vav[rt][:, :, 64], 1.0)
        nc.gpsimd.memset(vav[rt][:, :, 65:72], 0.0)

        for t in range(6):
            pe_transpose(kfT[t][:, rt * 128:(rt + 1) * 128],
                         kf[rt][:, t * 128:(t + 1) * 128], t)

    # ---- q-side (own tiles 0..1 after permutation) -------------------------
    def emit_qtile(nt):
        colbase = nt * 128
        ps_qp = ppsum.tile([128, WQP], F32, tag="proj", name="psqp")
        proj(ps_qp[:], wallQ, WQ_COLS, 0, WQP, colbase, WK_COLS)
        ps_qg = ppsum.tile([128, 240], F32, tag="proj", name="psqg")
        proj(ps_qg[:], wallQ, WQ_COLS, WQP, WQ_COLS, colbase, WK_COLS)

        qpts = tmp.tile([128, WQP], BF16, tag="qpts", name="qpts", bufs=2)
        nc.vector.tensor_scalar_max(qpts[:], ps_qp[:], 0.0)
        nc.scalar.copy(
            qfv[nt][:, :, 0:16],
            ps_qg[:, 0:192].rearrange("p (h c) -> p h c", c=16))
        nc.scalar.activation(gc_sb[nt][:], ps_qg[:, 192:240], AF.Sigmoid)
        nc.vector.tensor_scalar_mul(gd_sb[nt][:], gc_sb[nt][:],
                                    float(hp["gsc_di"]))
        nc.vector.tensor_scalar_mul(gc_sb[nt][:], gc_sb[nt][:],
                                    float(hp["gsc_co"]))
        return qpts

    def emit_qtransform(nt, qpts):
        own = nt
        rot = rt12[:, own * 12:own * 12 + 9]
        tr = rt12[:, own * 12 + 9:own * 12 + 12]
        Wq = H * P

        def qv(a, b):
            return qpts[:, a * Wq:b * Wq].rearrange("p (h x) -> p h x", x=P)

        for i in range(3):
            dco = qfv[nt][:, :, 16 + 4 * i:20 + 4 * i]
            nc.vector.tensor_scalar(dco, qv(0, 1), rot[:, 3 * i:3 * i + 1],
                                    tr[:, i:i + 1], OP.mult, OP.add)
            nc.vector.scalar_tensor_tensor(dco, qv(1, 2),
                                           rot[:, 3 * i + 1:3 * i + 2], dco,
                                           OP.mult, OP.add)
            nc.vector.scalar_tensor_tensor(dco, qv(2, 3),
                                           rot[:, 3 * i + 2:3 * i + 3], dco,
                                           OP.mult, OP.add)
            ddi = qfv[nt][:, :, 28 + 4 * i:32 + 4 * i]
            nc.vector.tensor_scalar_mul(ddi, qv(3, 4),
                                        rot[:, 3 * i:3 * i + 1])
            nc.vector.scalar_tensor_tensor(ddi, qv(4, 5),
                                           rot[:, 3 * i + 1:3 * i + 2], ddi,
                                           OP.mult, OP.add)
            nc.vector.scalar_tensor_tensor(ddi, qv(5, 6),
                                           rot[:, 3 * i + 2:3 * i + 3], ddi,
                                           OP.mult, OP.add)
        gcv = gc_sb[nt][:].rearrange("p (h x) -> p h x", x=P)
        gdv = gd_sb[nt][:].rearrange("p (h x) -> p h x", x=P)
        for i in range(3):
            sl = qfv[nt][:, :, 16 + 4 * i:20 + 4 * i]
            nc.vector.tensor_tensor(sl, sl, gcv, OP.mult)
            sl = qfv[nt][:, :, 28 + 4 * i:32 + 4 * i]
            nc.vector.tensor_tensor(sl, sl, gdv, OP.mult)

    def emit_qstats(nt):
        own = nt
        sq = tmp.tile([128, H * 12], F32, tag="qsq", name="qsq")
        qslots = qfv[nt][:, :, 16:28]
        nc.vector.tensor_tensor(sq[:].rearrange("p (h x) -> p h x", x=12),
                                qslots, qslots, OP.mult)
        q2 = tmp.tile([128, H], F32, tag="q2", name="q2")
        nc.vector.tensor_reduce(q2[:],
                                sq[:].rearrange("p (h x) -> p h x", x=12),
                                mybir.AxisListType.X, OP.add)

        kdv = kfv[own][:, :, 28:40]
        qdv = qfv[nt][:, :, 28:40]
        cs_ = tmp.tile([128, H * P], F32, tag="cs", name="cs")
        cr = tmp.tile([128, H * P], F32, tag="cr", name="cr")
        t3 = tmp.tile([128, H * P], F32, tag="t3", name="t3")
        crv = cr[:].rearrange("p (h x) -> p h x", x=P)
        t3v = t3[:].rearrange("p (h x) -> p h x", x=P)
        csv = cs_[:].rearrange("p (h x) -> p h x", x=P)
        first = True
        for (a, b_) in ((1, 2), (2, 0), (0, 1)):
            nc.gpsimd.tensor_tensor(crv, qdv[:, :, 4 * a:4 * a + 4],
                                    kdv[:, :, 4 * b_:4 * b_ + 4], OP.mult)
            nc.gpsimd.tensor_tensor(t3v, qdv[:, :, 4 * b_:4 * b_ + 4],
                                    kdv[:, :, 4 * a:4 * a + 4], OP.mult)
            nc.gpsimd.tensor_tensor(crv, crv, t3v, OP.subtract)
            nc.gpsimd.tensor_tensor(crv, crv, crv, OP.mult)
            if first:
                nc.gpsimd.tensor_copy(csv, crv)
                first = False
            else:
                nc.gpsimd.tensor_tensor(csv, csv, crv, OP.add)
        sqd = tmp.tile([128, H * 12], F32, tag="sqd", name="sqd")
        nq2 = tmp.tile([128, H * P], F32, tag="nq2", name="nq2")
        nk2 = tmp.tile([128, H * P], F32, tag="nk2", name="nk2")
        nc.vector.tensor_tensor(sqd[:].rearrange("p (h y) -> p h y", y=12),
                                qdv, qdv, OP.mult)
        nc.vector.tensor_reduce(
            nq2[:].rearrange("p (h x) -> p h x", x=P),
            sqd[:].rearrange("p (h i x) -> p h x i", i=3, x=P),
            mybir.AxisListType.X, OP.add)
        nc.vector.tensor_tensor(sqd[:].rearrange("p (h y) -> p h y", y=12),
                                kdv, kdv, OP.mult)
        nc.vector.tensor_reduce(
            nk2[:].rearrange("p (h x) -> p h x", x=P),
            sqd[:].rearrange("p (h i x) -> p h x i", i=3, x=P),
            mybir.AxisListType.X, OP.add)
        nc.gpsimd.tensor_tensor(nq2[:], nq2[:], nk2[:], OP.mult)
        nc.scalar.activation(nq2[:], nq2[:], AF.Sqrt)      # |qd||kd|
        nc.vector.tensor_scalar_add(nq2[:], nq2[:], EPS)
        nc.vector.reciprocal(nq2[:], nq2[:])
        nc.scalar.activation(cs_[:], cs_[:], AF.Sqrt)      # |cross|
        nc.gpsimd.tensor_tensor(cs_[:], cs_[:], nq2[:], OP.mult)
        curv = tmp.tile([128, H], F32, tag="curv", name="curv")
        nc.vector.tensor_reduce(curv[:], csv, mybir.AxisListType.X, OP.add)
        nc.vector.tensor_scalar_mul(q2[:], q2[:], float(hp["c_q2"]))
        nc.vector.scalar_tensor_tensor(q2[:], curv[:], float(hp["c_curv"]),
                                       q2[:], OP.mult, OP.add)
        nc.vector.tensor_copy(qfv[nt][:, :, 41], q2[:])
        nc.gpsimd.memset(qfv[nt][:, :, 40], 1.0)
        nc.gpsimd.memset(qfv[nt][:, :, 42:64], 0.0)
        for t in range(6):
            pe_transpose(qfT[t][:, nt * 128:(nt + 1) * 128],
                         qf[nt][:, t * 128:(t + 1) * 128], t)

    # ---------------- emission order --------------------------------------
    qpts_pend = {}
    for rt in range(RT):
        emit_ktile(rt)
        if rt == 1:
            for nt in range(NT):
                qpts_pend[nt] = emit_qtile(nt)
                emit_qtransform(nt, qpts_pend[nt])
            for nt in range(NT):
                emit_qstats(nt)

    ppool.close()

    # ---- attention --------------------------------------------------------
    att_ctx = ExitStack()
    apsum = att_ctx.enter_context(tc.tile_pool(name="apsum", bufs=2, space=PS))
    opsum = att_ctx.enter_context(tc.tile_pool(name="opsum", bufs=2, space=PS))
    expT_tiles = [work.tile([128, 2 * N], BF16, name=f"expT{i}")
                  for i in range(4)]
    o_all = [work.tile([128, FEAT * H], BF16, name=f"oall{qt}")
             for qt in range(NT)]
    feats = [work.tile([128, FPAD], BF16, name=f"feats{qt}")
             for qt in range(NT)]
    for qt in range(NT):
        nc.gpsimd.memset(feats[qt][:, FOUT:FOUT + 2], 1.0)
        nc.gpsimd.memset(feats[qt][:, FOUT + 2:], 0.0)
    RUN = 3

    def emit_qk_exp(h):
        t, base = h // 2, (h % 2) * FEAT
        expT = expT_tiles[h % 4]
        for half in range(2):
            aps = apsum.tile([128, 4 * NB], F32, tag="attT", name="aps")
            for kb4 in range(4):
                kb = half * 4 + kb4
                nc.tensor.matmul(
                    aps[:, kb4 * NB:(kb4 + 1) * NB],
                    kfT[t][base:base + FS, kb * 128:(kb + 1) * 128],
                    qfT[t][base:base + FS, :],
                    start=True, stop=True)
            nc.scalar.activation(expT[:, half * 4 * NB:(half + 1) * 4 * NB],
                                 aps[:], AF.Exp, scale=float(sh[h]))

    def emit_av(h):
        expT = expT_tiles[h % 4]
        ot_ps = opsum.tile([VLD, NB], F32, tag="otacc", name="ot_ps")
        for kb in range(8):
            nc.tensor.matmul(
                ot_ps[:],
                va[kb][:, h * VLD:(h + 1) * VLD],
                expT[:, kb * NB:(kb + 1) * NB],
                start=(kb == 0), stop=(kb == 7))
        ot_sb = tmp.tile([VLD, NB], F32R, tag="otsb", name="otsb", bufs=2)
        nc.scalar.copy(ot_sb[:], ot_ps[:])
        for qt in range(NT):
            tp = opsum.tile([128, VLD], F32R, tag="otp", name="tp")
            nc.tensor.transpose(tp[:], ot_sb[:, qt * 128:(qt + 1) * 128],
                                ident_r[:])
            rec = tmp.tile([128, 1], F32, tag="rec", name="rec")
            nc.vector.reciprocal(rec[:], tp[:, 64:65].bitcast(F32))
            nc.vector.tensor_scalar_mul(
                o_all[qt][:, h * FEAT:h * FEAT + 64],
                tp[:, 0:64].bitcast(F32), rec[:])

    GH = 6                   # heads per inverse-transform group

    def emit_inverse(qt, g):
        own = qt
        rot = rt12[:, own * 12:own * 12 + 9]
        tr = rt12[:, own * 12 + 9:own * 12 + 12]
        hs = slice(g * GH, g * GH + GH)
        ovv = o_all[qt][:].rearrange("p (h f) -> p h f", f=FEAT)[:, hs]

        def og(j):  # [128, 6, V] component j of attention-weighted points
            return ovv[:, :, 16 + 8 * j:24 + 8 * j]

        nc.vector.tensor_copy(
            feats[qt][:, g * GH * 16:(g + 1) * GH * 16].rearrange(
                "p (h c) -> p h c", c=16),
            ovv[:, :, 0:16])
        gview = feats[qt][:, 192:FOUT].rearrange(
            "p (h x c) -> p h x c", h=H, c=7)[:, hs]

        ogs = [tmp.tile([128, GH * V], BF16, tag=f"ogs{j}", name=f"ogs{j}")
               for j in range(3)]
        for j in range(3):
            nc.vector.tensor_scalar(
                ogs[j][:].rearrange("p (h x) -> p h x", x=V), og(j),
                tr[:, j:j + 1], None, OP.subtract)
        lc = [tmp.tile([128, GH * V], BF16, tag=f"lc{i}", name=f"lc{i}")
              for i in range(3)]
        ld = [tmp.tile([128, GH * V], BF16, tag=f"ld{i}", name=f"ld{i}")
              for i in range(3)]
        for i in range(3):
            nc.vector.tensor_scalar_mul(lc[i][:], ogs[0][:], rot[:, i:i + 1])
            nc.vector.scalar_tensor_tensor(lc[i][:], ogs[1][:],
                                           rot[:, 3 + i:4 + i],
                                           lc[i][:], OP.mult, OP.add)
            nc.vector.scalar_tensor_tensor(lc[i][:], ogs[2][:],
                                           rot[:, 6 + i:7 + i],
                                           lc[i][:], OP.mult, OP.add)
            ldv = ld[i][:].rearrange("p (h x) -> p h x", x=V)
            nc.vector.tensor_scalar_mul(ldv, og(3), rot[:, i:i + 1])
            nc.vector.scalar_tensor_tensor(ldv, og(4), rot[:, 3 + i:4 + i],
                                           ldv, OP.mult, OP.add)
            nc.vector.scalar_tensor_tensor(ldv, og(5), rot[:, 6 + i:7 + i],
                                           ldv, OP.mult, OP.add)
        n2 = tmp.tile([128, GH * V], F32, tag="n2", name="n2")
        t2b = tmp.tile([128, GH * V], F32, tag="t2b", name="t2b")
        nc.vector.tensor_tensor(n2[:], lc[0][:], lc[0][:], OP.mult)
        for i in (1, 2):
            nc.vector.tensor_tensor(t2b[:], lc[i][:], lc[i][:], OP.mult)
            nc.vector.tensor_tensor(n2[:], n2[:], t2b[:], OP.add)
        nc.scalar.activation(
            gview[:, :, :, 6].rearrange("p h x -> p (h x)"), n2[:], AF.Sqrt)
        for i in range(3):
            nc.vector.tensor_copy(
                gview[:, :, :, i].rearrange("p h x -> p (h x)"), lc[i][:])
        nc.vector.tensor_tensor(n2[:], ld[0][:], ld[0][:], OP.mult)
        for i in (1, 2):
            nc.vector.tensor_tensor(t2b[:], ld[i][:], ld[i][:], OP.mult)
            nc.vector.tensor_tensor(n2[:], n2[:], t2b[:], OP.add)
        nc.scalar.activation(n2[:], n2[:], AF.Sqrt)
        nc.vector.tensor_scalar_max(n2[:], n2[:], EPS)
        nc.vector.reciprocal(n2[:], n2[:])
        rn2 = tmp.tile([128, GH * V], BF16, tag="rn2", name="rn2")
        nc.vector.tensor_copy(rn2[:], n2[:])
        for i in range(3):
            nc.vector.tensor_tensor(
                gview[:, :, :, 3 + i].rearrange("p h x -> p (h x)"),
                ld[i][:], rn2[:], OP.mult)

    for h in range(H + RUN):
        if h < H:
            emit_qk_exp(h)
        if h >= RUN:
            emit_av(h - RUN)
            if h - RUN == 5:
                for qt in range(NT):
                    emit_inverse(qt, 0)
    for qt in range(NT):
        emit_inverse(qt, 1)

    att_ctx.close()

    # ---- output projection (feats -> DRAM -> xbar transpose -> matmul) ---
    opool = ExitStack()
    opsum2 = opool.enter_context(tc.tile_pool(name="opsum2", bufs=2, space=PS))
    fT = [work.tile([128, NB], BF16, name=f"fT{kc}") for kc in range(KCH)]
    for kc in range(KCH):
        for qt in range(NT):
            ps = opsum2.tile([128, 128], BF16, tag="ftps", name="ftps")
            nc.tensor.transpose(ps[:], feats[qt][:, kc * 128:(kc + 1) * 128],
                                identb[:])
            if kc % 2:
                nc.scalar.copy(fT[kc][:, qt * 128:(qt + 1) * 128], ps[:])
            else:
                nc.vector.tensor_copy(fT[kc][:, qt * 128:(qt + 1) * 128],
                                      ps[:])

    for qt in range(NT):
        ps = opsum2.tile([128, CS], F32, tag="oproj")
        for kc in range(KCH):
            nc.tensor.matmul(ps[:], fT[kc][:, qt * 128:(qt + 1) * 128],
                             woutp[:, kc * CS:(kc + 1) * CS],
                             start=(kc == 0), stop=(kc == KCH - 1))
        osb = tmp.tile([128, CS], F32, tag="osb", name="osb")
        nc.scalar.copy(osb[:], ps[:])
        nc.sync.dma_start(out_d[qt * 128:(qt + 1) * 128, :], osb[:])
    opool.close()


def _run(inputs, trace=False):
    hp = _host_prep(inputs)
    nc = _build_program(hp)
    in_maps = []
    for c in range(8):
        m = {
            "sT": hp["sT"][c], "rt12": hp["rt12"][c],
            "wallK": hp["wallK"], "wallQ": hp["wallQ"],
            "woutp": hp["woutp"],
        }
        if hp["has_bias"]:
            m["wbias"] = hp["wbias"]
        in_maps.append(m)
    res = run_bass_kernel_spmd(nc, in_maps, list(range(8)), trace=trace)
    out = np.empty((B, N, CS), np.float32)
    for c in range(8):
        b, qb = c // 4, c % 4
        out[b, qb * NB:(qb + 1) * NB] = res.results[c]["out_loc"]
    return out, res


def kernel(**inputs):
    out, _ = _run(inputs, trace=False)
    return out


def kernel_traced(**inputs):
    return _run(inputs, trace=True)


# revision 27
# speedup vs baseline: 1.0100x; 1.0100x over previous
"""Bass/Tile TRN2 kernel for EnhancedIPA3 (invariant-point-attention variant).

v3 strategy: 8 cores = batch(2) x query-block(4), **no collectives**.
Each core receives the FULL (host-transposed, bf16) s for its batch and
computes K-side features for all 1024 keys locally; only Q-side work and
attention are sharded by query block (key rows are host-permuted so each
core's own block sits at tiles 0-1, letting one SPMD program serve all
cores).  Feature transposes ride the DMA xbar (SBUF->DRAM->SBUF
dma_start_transpose).  Per-head softmax temperature (sigmoid(head_w)) is
folded into the EXP activation scale; q-feature scales are folded into
wq / the gates, eliminating the qscale multiply.

Self-contained: hardcodes all shapes; only depends on numpy + concourse.
"""

import numpy as np
import ml_dtypes
from contextlib import ExitStack

import concourse.bass as bass
import concourse.bacc as bacc
import concourse.mybir as mybir
import concourse.tile as tile
from concourse.bass_utils import run_bass_kernel_spmd
from concourse.masks import make_identity

F32 = mybir.dt.float32
F32R = mybir.dt.float32r
BF16 = mybir.dt.bfloat16
AF = mybir.ActivationFunctionType
OP = mybir.AluOpType

B, N, CS, H, C, P, V = 2, 1024, 384, 12, 16, 4, 8
EPS = 1e-8
NB = N // 4              # 256 query rows per core
NT = NB // 128           # 2 query row-tiles per core
RT = N // 128            # 8 key row-tiles (all computed locally)
KVP = P + V              # 12 kv points per head
FEAT = 64                # padded per-head attention feature stride
FS = 42                  # used attention features per head
FOUT = H * (C + 7 * V)   # 864 output-proj input channels
FPAD = 896               # feats padded to 7*128 (ones at 864:866, zero pad)
KCH = 7                  # contraction chunks for output proj
VLD = 72                 # per-head va block: vs 16 | 6 comps x 8 | ones | pad7
VCOLS = H * VLD          # 864 va columns

# wallK columns: per comp j: [kp (h,4)=48 | vp (h,8)=96]; then k, v scalars
WKP = 6 * H * KVP        # 864
WK_COLS = WKP + 192 + 192            # 1248
# wallQ columns: [q-pts comp-major 6*48 | q (h,c) 192 | g (h,4) 48]
WQP = 6 * H * P          # 288
WQ_COLS = WQP + 192 + 48             # 528


def _host_prep(inputs):
    """Layout-only host prep: transposes, dtype casts, col permutations,
    and folding of per-head scales into weights/gates/exp-scale."""
    wq = np.asarray(inputs["wq"], np.float32)
    wkv = np.asarray(inputs["wkv"], np.float32)
    wqp = np.asarray(inputs["wqp"], np.float32)
    wkvp = np.asarray(inputs["wkvp"], np.float32)
    wg = np.asarray(inputs["wg"], np.float32)
    biases = [np.asarray(inputs[k], np.float32)
              for k in ("bq", "bkv", "bqp", "bkvp", "bg")]
    has_bias = any(np.abs(b).max() > 0 for b in biases)
    bq, bkv, bqp, bkvp, bg = biases
    gw = np.asarray(inputs["geom_weight"], np.float32)
    hw = np.asarray(inputs["head_weights"], np.float32)
    sh = 1.0 / (1.0 + np.exp(-hw))           # sigmoid(head_weights) [H]

    # ---- wallK [384+1, 1248] ----
    wallK = np.zeros((CS + 1, WK_COLS), np.float32)
    wkvp_r = wkvp.reshape(CS, H, KVP, 6)
    bkvp_r = bkvp.reshape(H, KVP, 6)
    for j in range(6):
        o = j * 144
        wallK[:CS, o:o + 48] = wkvp_r[:, :, :P, j].reshape(CS, 48)
        wallK[CS, o:o + 48] = bkvp_r[:, :P, j].reshape(-1)
        wallK[:CS, o + 48:o + 144] = wkvp_r[:, :, P:, j].reshape(CS, 96)
        wallK[CS, o + 48:o + 144] = bkvp_r[:, P:, j].reshape(-1)
    wallK[:CS, WKP:WKP + 192] = wkv[:, :192]
    wallK[CS, WKP:WKP + 192] = bkv[:192]
    wallK[:CS, WKP + 192:] = wkv[:, 192:]
    wallK[CS, WKP + 192:] = bkv[192:]

    # ---- wallQ [384+1, 528]; wq pre-scaled by 1/sqrt(C) ----
    wallQ = np.zeros((CS + 1, WQ_COLS), np.float32)
    wqp_r = wqp.reshape(CS, H, P, 6)
    bqp_r = bqp.reshape(H, P, 6)
    for j in range(6):
        wallQ[:CS, j * H * P:(j + 1) * H * P] = \
            wqp_r[:, :, :, j].reshape(CS, H * P)
        wallQ[CS, j * H * P:(j + 1) * H * P] = bqp_r[:, :, j].reshape(-1)
    cs_scale = 1.0 / np.sqrt(C)
    wallQ[:CS, WQP:WQP + 192] = wq * cs_scale
    wallQ[CS, WQP:WQP + 192] = bq * cs_scale
    wallQ[:CS, WQP + 192:] = wg
    wallQ[CS, WQP + 192:] = bg

    def pack_chunks(w):
        cols = w.shape[1]
        out = np.zeros((128, 3, cols), np.float32)
        for kc in range(3):
            out[:, kc, :] = w[kc * 128:(kc + 1) * 128, :]
        return out.reshape(128, -1).astype(ml_dtypes.bfloat16)

    wallK_p = pack_chunks(wallK[:CS])
    wallQ_p = pack_chunks(wallQ[:CS])
    wbias = np.concatenate([wallK[CS:], wallQ[CS:]],
                           axis=1).astype(ml_dtypes.bfloat16)  # [1, 1776]

    # ---- wout packed [128, 7, 384] ----
    wout = np.asarray(inputs["wout"], np.float32)
    bout_half = np.asarray(inputs["bout"], np.float32)[None, :] * 0.5
    wout_b = np.concatenate([wout, bout_half, bout_half], axis=0)  # [866,384]
    woutp = np.zeros((128, KCH, CS), np.float32)
    for kc in range(KCH):
        r0 = kc * 128
        r1 = min(866, r0 + 128)
        woutp[:r1 - r0, kc, :] = wout_b[r0:r1]
    woutp = woutp.reshape(128, -1).astype(ml_dtypes.bfloat16)

    # gate scales & combo-column coefficients (qscale elimination)
    gsc_co = gw[0] * 0.5 if gw[0] != 0 else 1.0   # coord-slot gate scale
    gsc_di = gw[1] if gw[1] != 0 else 1.0          # dir-slot gate scale
    c_q2 = (-gw[0] / P) / (gsc_co * gsc_co) if gw[0] != 0 else 0.0
    c_curv = -gw[1] / P

    # ---- per-core: sT packed + rot/trans (rows permuted, own block first)
    s = np.asarray(inputs["s"], np.float32)
    rot = np.asarray(inputs["rot"], np.float32).reshape(B, N, 9)
    trans = np.asarray(inputs["trans"], np.float32)
    sT_p, rt12_p = [], []
    for c in range(8):
        b, qb = c // 4, c % 4
        perm = [qb] + [x for x in range(4) if x != qb]
        ridx = np.concatenate([np.arange(p * NB, (p + 1) * NB) for p in perm])
        sT = np.ascontiguousarray(s[b][ridx].T)    # [384, 1024]
        stp = sT.reshape(3, 128, N).transpose(1, 0, 2).reshape(128, 3 * N)
        sT_p.append(stp.astype(ml_dtypes.bfloat16))
        rt = np.concatenate([rot[b][ridx], trans[b][ridx]], axis=1)
        rtp = rt.reshape(RT, 128, 12).transpose(1, 0, 2).reshape(128, RT * 12)
        rt12_p.append(rtp.astype(np.float32))

    return dict(wallK=wallK_p, wallQ=wallQ_p, wbias=wbias, woutp=woutp,
                sT=sT_p, rt12=rt12_p, gw=gw, sh=sh, has_bias=has_bias,
                gsc_co=gsc_co, gsc_di=gsc_di, c_q2=c_q2, c_curv=c_curv)


_PROGRAM_CACHE = {}


def _build_program(hp):
    key = (tuple(np.round(hp["sh"], 7).tolist()), float(hp["gw"][0]),
           float(hp["gw"][1]), bool(hp["has_bias"]))
    if key in _PROGRAM_CACHE:
        return _PROGRAM_CACHE[key]

    nc = bacc.Bacc("TRN2", target_bir_lowering=False, debug=False,
                   num_devices=8)

    sT_d = nc.dram_tensor("sT", [128, 3 * N], BF16, kind="ExternalInput")
    rt12_d = nc.dram_tensor("rt12", [128, RT * 12], F32, kind="ExternalInput")
    wallK_d = nc.dram_tensor("wallK", [128, 3 * WK_COLS], BF16,
                             kind="ExternalInput")
    wallQ_d = nc.dram_tensor("wallQ", [128, 3 * WQ_COLS], BF16,
                             kind="ExternalInput")
    woutp_d = nc.dram_tensor("woutp", [128, KCH * CS], BF16,
                             kind="ExternalInput")
    wbias_d = nc.dram_tensor("wbias", [1, WK_COLS + WQ_COLS], BF16,
                             kind="ExternalInput") if hp["has_bias"] else None
    out_d = nc.dram_tensor("out_loc", [NB, CS], F32, kind="ExternalOutput")

    with tile.TileContext(nc) as tc:
        with ExitStack() as ctx:
            _emit(ctx, tc, nc, sT_d, rt12_d, wallK_d, wallQ_d, wbias_d,
                  woutp_d, out_d, hp)

    nc.compile()
    _PROGRAM_CACHE[key] = nc
    return nc


def _emit(ctx, tc, nc, sT_d, rt12_d, wallK_d, wallQ_d, wbias_d, woutp_d,
          out_d, hp):
    PS = bass.MemorySpace.PSUM
    gw0, gw1 = float(hp["gw"][0]), float(hp["gw"][1])
    sh = hp["sh"]
    has_bias = hp["has_bias"]

    const = ctx.enter_context(tc.tile_pool(name="const", bufs=1))
    work = ctx.enter_context(tc.tile_pool(name="work", bufs=1))
    tmp = ctx.enter_context(tc.tile_pool(name="tmp", bufs=2))

    # ---- constant loads ---------------------------------------------------
    sT_sb = const.tile([128, 3 * N], BF16, name="sT")
    for kc in range(3):
        nc.sync.dma_start(sT_sb[:, kc * N:(kc + 1) * N],
                          sT_d[:, kc * N:(kc + 1) * N])
    rt12 = const.tile([128, RT * 12], F32, name="rt12")
    nc.sync.dma_start(rt12[:], rt12_d[:, :])
    wallK = const.tile([128, 3 * WK_COLS], BF16, name="wallK")
    for kc in range(3):
        nc.sync.dma_start(wallK[:, kc * WK_COLS:(kc + 1) * WK_COLS],
                          wallK_d[:, kc * WK_COLS:(kc + 1) * WK_COLS])
    wallQ = const.tile([128, 3 * WQ_COLS], BF16, name="wallQ")
    nc.sync.dma_start(wallQ[:], wallQ_d[:, :])
    woutp = const.tile([128, KCH * CS], BF16, name="woutp")
    nc.sync.dma_start(woutp[:], woutp_d[:, :])
    if has_bias:
        wbias = const.tile([1, WK_COLS + WQ_COLS], BF16, name="wbias")
        nc.sync.dma_start(wbias[:], wbias_d[:, :])
        ones1 = const.tile([1, N], BF16, name="ones1")
        nc.gpsimd.memset(ones1[:], 1.0)

    ident = const.tile([128, 128], F32, name="ident")
    make_identity(nc, ident[:])
    ident_r = const.tile([VLD, VLD], F32R, name="identr")
    nc.vector.tensor_copy(ident_r[:], ident[0:VLD, 0:VLD])
    identb = const.tile([128, 128], BF16, name="identb")
    nc.vector.tensor_copy(identb[:], ident[:])
    # pin the sigmoid table set before any relu evacuations
    actpin = const.tile([1, 1], F32, name="actpin")
    nc.scalar.activation(actpin[:], ident[0:1, 0:1], AF.Sigmoid)

    # ---- persistent feature tiles ----------------------------------------
    kf = [work.tile([128, FEAT * H], BF16, name=f"kf{rt}") for rt in range(RT)]
    va = [work.tile([128, VCOLS], BF16, name=f"va{rt}") for rt in range(RT)]
    qf = [work.tile([128, FEAT * H], BF16, name=f"qf{nt}") for nt in range(NT)]
    gc_sb = [work.tile([128, H * P], BF16, name=f"gc{nt}") for nt in range(NT)]
    gd_sb = [work.tile([128, H * P], BF16, name=f"gd{nt}") for nt in range(NT)]

    kfT = [work.tile([128, N], BF16, name=f"kfT{t}") for t in range(6)]
    qfT = [work.tile([128, NB], BF16, name=f"qfT{t}") for t in range(6)]
    kfv = [t[:].rearrange("p (h f) -> p h f", f=FEAT) for t in kf]
    vav = [t[:].rearrange("p (h f) -> p h f", f=VLD) for t in va]
    qfv = [t[:].rearrange("p (h f) -> p h f", f=FEAT) for t in qf]

    ppool = ExitStack()
    ppsum = ppool.enter_context(tc.tile_pool(name="ppsum", bufs=3, space=PS))
    tpsum = ppool.enter_context(tc.tile_pool(name="tpsum", bufs=2, space=PS))
    KC = 3

    def pe_transpose(dst, src_ap, t_idx):
        ps = tpsum.tile([128, 128], BF16, tag="tps", name="tps")
        nc.tensor.transpose(ps[:], src_ap, identb[:])
        if t_idx % 2:
            nc.scalar.copy(dst, ps[:])
        else:
            nc.vector.tensor_copy(dst, ps[:])

    def proj(psv, wall_sb, wcols, c0, c1, colbase, bias_off):
        for kc in range(KC):
            nc.tensor.matmul(
                psv,
                sT_sb[:, kc * N + colbase:kc * N + colbase + 128],
                wall_sb[:, kc * wcols + c0:kc * wcols + c1],
                start=(kc == 0), stop=(kc == KC - 1 and not has_bias))
        if has_bias:
            nc.tensor.matmul(psv, ones1[:, colbase:colbase + 128],
                             wbias[:, bias_off + c0:bias_off + c1],
                             start=False, stop=True)

    def emit_ktile(rt):
        colbase = rt * 128
        rot = rt12[:, rt * 12:rt * 12 + 9]
        tr = rt12[:, rt * 12 + 9:rt * 12 + 12]
        W = H * KVP  # 144
        fma_eng = nc.vector

        ps_co = ppsum.tile([128, 3 * W], F32, tag="proj", name="psco")
        proj(ps_co[:], wallK, WK_COLS, 0, 3 * W, colbase, 0)
        ps_di = ppsum.tile([128, 3 * W], F32, tag="proj", name="psdi")
        proj(ps_di[:], wallK, WK_COLS, 3 * W, WKP, colbase, 0)
        ps_kv = ppsum.tile([128, 384], F32, tag="proj", name="pskv")
        proj(ps_kv[:], wallK, WK_COLS, WKP, WK_COLS, colbase, 0)

        # evacuate: relu pts on DVE; k scalars ACT; v scalars ACT (va 0:192)
        pts = tmp.tile([128, WKP], BF16, tag="pts", name="pts", bufs=3)
        nc.scalar.activation(pts[:, 0:3 * W], ps_co[:], AF.Relu)
        nc.vector.tensor_scalar_max(pts[:, 3 * W:6 * W], ps_di[:], 0.0)
        nc.scalar.copy(
            kfv[rt][:, :, 0:16],
            ps_kv[:, 0:192].rearrange("p (h c) -> p h c", c=16))
        nc.scalar.copy(
            vav[rt][:, :, 0:16],
            ps_kv[:, 192:384].rearrange("p (h c) -> p h c", c=16))

        # rigid transform into pco: init on gpsimd, fma on DVE -------------
        pco = tmp.tile([128, WKP], BF16, tag="pco", name="pco", bufs=3)
        for i in range(3):
            dco = pco[:, i * W:(i + 1) * W]
            nc.scalar.activation(dco, pts[:, 0:W], AF.Identity,
                                 bias=tr[:, i:i + 1],
                                 scale=rot[:, 3 * i:3 * i + 1])
            fma_eng.scalar_tensor_tensor(dco, pts[:, W:2 * W],
                                         rot[:, 3 * i + 1:3 * i + 2], dco,
                                         OP.mult, OP.add)
            fma_eng.scalar_tensor_tensor(dco, pts[:, 2 * W:3 * W],
                                         rot[:, 3 * i + 2:3 * i + 3], dco,
                                         OP.mult, OP.add)
            ddi = pco[:, (3 + i) * W:(4 + i) * W]
            nc.scalar.activation(ddi, pts[:, 3 * W:4 * W], AF.Identity,
                                 scale=rot[:, 3 * i:3 * i + 1])
            fma_eng.scalar_tensor_tensor(ddi, pts[:, 4 * W:5 * W],
                                         rot[:, 3 * i + 1:3 * i + 2], ddi,
                                         OP.mult, OP.add)
            fma_eng.scalar_tensor_tensor(ddi, pts[:, 5 * W:6 * W],
                                         rot[:, 3 * i + 2:3 * i + 3], ddi,
                                         OP.mult, OP.add)

        # kp -> kf slots (ACT, strided); vp -> va groups (gpsimd, contig) --
        for j in range(6):
            nc.scalar.copy(
                kfv[rt][:, :, 16 + 4 * j:20 + 4 * j],
                pco[:, j * W:j * W + 48].rearrange("p (h x) -> p h x", x=P))
            nc.vector.tensor_copy(
                vav[rt][:, :, 16 + 8 * j:24 + 8 * j],
                pco[:, j * W + 48:(j + 1) * W].rearrange(
                    "p (h x) -> p h x", x=V))

        # k2 term -> kf col 40 ----------------------------------------------
        sq = tmp.tile([128, H * 12], F32, tag="sq", name="sq")
        cslots = kfv[rt][:, :, 16:28]
        nc.vector.tensor_tensor(sq[:].rearrange("p (h x) -> p h x", x=12),
                                cslots, cslots, OP.mult)
        k2 = tmp.tile([128, H], F32, tag="k2", name="k2")
        nc.vector.tensor_reduce(k2[:],
                                sq[:].rearrange("p (h x) -> p h x", x=12),
                                mybir.AxisListType.X, OP.add)
        nc.vector.tensor_scalar_mul(kfv[rt][:, :, 40], k2[:], -gw0 / P)
        nc.gpsimd.memset(kfv[rt][:, :, 41], 1.0)
        nc.gpsimd.memset(kfv[rt][:, :, 42:64], 0.0)
        nc.gpsimd.memset(vav[rt][:, :, 64], 1.0)
        nc.gpsimd.memset(vav[rt][:, :, 65:72], 0.0)

        for t in range(6):
            pe_transpose(kfT[t][:, rt * 128:(rt + 1) * 128],
                         kf[rt][:, t * 128:(t + 1) * 128], t)

    # ---- q-side (own tiles 0..1 after permutation) -------------------------
    def emit_qtile(nt):
        colbase = nt * 128
        ps_qp = ppsum.tile([128, WQP], F32, tag="proj", name="psqp")
        proj(ps_qp[:], wallQ, WQ_COLS, 0, WQP, colbase, WK_COLS)
        ps_qg = ppsum.tile([128, 240], F32, tag="proj", name="psqg")
        proj(ps_qg[:], wallQ, WQ_COLS, WQP, WQ_COLS, colbase, WK_COLS)

        qpts = tmp.tile([128, WQP], BF16, tag="qpts", name="qpts", bufs=2)
        nc.vector.tensor_scalar_max(qpts[:], ps_qp[:], 0.0)
        nc.scalar.copy(
            qfv[nt][:, :, 0:16],
            ps_qg[:, 0:192].rearrange("p (h c) -> p h c", c=16))
        nc.scalar.activation(gc_sb[nt][:], ps_qg[:, 192:240], AF.Sigmoid)
        nc.vector.tensor_scalar_mul(gd_sb[nt][:], gc_sb[nt][:],
                                    float(hp["gsc_di"]))
        nc.vector.tensor_scalar_mul(gc_sb[nt][:], gc_sb[nt][:],
                                    float(hp["gsc_co"]))
        return qpts

    def emit_qtransform(nt, qpts):
        own = nt
        rot = rt12[:, own * 12:own * 12 + 9]
        tr = rt12[:, own * 12 + 9:own * 12 + 12]
        Wq = H * P

        def qv(a, b):
            return qpts[:, a * Wq:b * Wq].rearrange("p (h x) -> p h x", x=P)

        for i in range(3):
            dco = qfv[nt][:, :, 16 + 4 * i:20 + 4 * i]
            nc.vector.tensor_scalar(dco, qv(0, 1), rot[:, 3 * i:3 * i + 1],
                                    tr[:, i:i + 1], OP.mult, OP.add)
            nc.vector.scalar_tensor_tensor(dco, qv(1, 2),
                                           rot[:, 3 * i + 1:3 * i + 2], dco,
                                           OP.mult, OP.add)
            nc.vector.scalar_tensor_tensor(dco, qv(2, 3),
                                           rot[:, 3 * i + 2:3 * i + 3], dco,
                                           OP.mult, OP.add)
            ddi = qfv[nt][:, :, 28 + 4 * i:32 + 4 * i]
            nc.vector.tensor_scalar_mul(ddi, qv(3, 4),
                                        rot[:, 3 * i:3 * i + 1])
            nc.vector.scalar_tensor_tensor(ddi, qv(4, 5),
                                           rot[:, 3 * i + 1:3 * i + 2], ddi,
                                           OP.mult, OP.add)
            nc.vector.scalar_tensor_tensor(ddi, qv(5, 6),
                                           rot[:, 3 * i + 2:3 * i + 3], ddi,
                                           OP.mult, OP.add)
        gcv = gc_sb[nt][:].rearrange("p (h x) -> p h x", x=P)
        gdv = gd_sb[nt][:].rearrange("p (h x) -> p h x", x=P)
        for i in range(3):
            sl = qfv[nt][:, :, 16 + 4 * i:20 + 4 * i]
            nc.vector.tensor_tensor(sl, sl, gcv, OP.mult)
            sl = qfv[nt][:, :, 28 + 4 * i:32 + 4 * i]
            nc.vector.tensor_tensor(sl, sl, gdv, OP.mult)

    def emit_qstats(nt):
        own = nt
        sq = tmp.tile([128, H * 12], F32, tag="qsq", name="qsq")
        qslots = qfv[nt][:, :, 16:28]
        nc.vector.tensor_tensor(sq[:].rearrange("p (h x) -> p h x", x=12),
                                qslots, qslots, OP.mult)
        q2 = tmp.tile([128, H], F32, tag="q2", name="q2")
        nc.vector.tensor_reduce(q2[:],
                                sq[:].rearrange("p (h x) -> p h x", x=12),
                                mybir.AxisListType.X, OP.add)

        kdv = kfv[own][:, :, 28:40]
        qdv = qfv[nt][:, :, 28:40]
        cs_ = tmp.tile([128, H * P], F32, tag="cs", name="cs")
        cr = tmp.tile([128, H * P], F32, tag="cr", name="cr")
        t3 = tmp.tile([128, H * P], F32, tag="t3", name="t3")
        crv = cr[:].rearrange("p (h x) -> p h x", x=P)
        t3v = t3[:].rearrange("p (h x) -> p h x", x=P)
        csv = cs_[:].rearrange("p (h x) -> p h x", x=P)
        first = True
        for (a, b_) in ((1, 2), (2, 0), (0, 1)):
            nc.gpsimd.tensor_tensor(crv, qdv[:, :, 4 * a:4 * a + 4],
                                    kdv[:, :, 4 * b_:4 * b_ + 4], OP.mult)
            nc.gpsimd.tensor_tensor(t3v, qdv[:, :, 4 * b_:4 * b_ + 4],
                                    kdv[:, :, 4 * a:4 * a + 4], OP.mult)
            nc.gpsimd.tensor_tensor(crv, crv, t3v, OP.subtract)
            nc.gpsimd.tensor_tensor(crv, crv, crv, OP.mult)
            if first:
                nc.gpsimd.tensor_copy(csv, crv)
                first = False
            else:
                nc.gpsimd.tensor_tensor(csv, csv, crv, OP.add)
        sqd = tmp.tile([128, H * 12], F32, tag="sqd", name="sqd")
        nq2 = tmp.tile([128, H * P], F32, tag="nq2", name="nq2")
        nk2 = tmp.tile([128, H * P], F32, tag="nk2", name="nk2")
        nc.vector.tensor_tensor(sqd[:].rearrange("p (h y) -> p h y", y=12),
                                qdv, qdv, OP.mult)
        nc.vector.tensor_reduce(
            nq2[:].rearrange("p (h x) -> p h x", x=P),
            sqd[:].rearrange("p (h i x) -> p h x i", i=3, x=P),
            mybir.AxisListType.X, OP.add)
        nc.vector.tensor_tensor(sqd[:].rearrange("p (h y) -> p h y", y=12),
                                kdv, kdv, OP.mult)
        nc.vector.tensor_reduce(
            nk2[:].rearrange("p (h x) -> p h x", x=P),
            sqd[:].rearrange("p (h i x) -> p h x i", i=3, x=P),
            mybir.AxisListType.X, OP.add)
        nc.gpsimd.tensor_tensor(nq2[:], nq2[:], nk2[:], OP.mult)
        nc.scalar.activation(nq2[:], nq2[:], AF.Sqrt)      # |qd||kd|
        nc.vector.tensor_scalar_add(nq2[:], nq2[:], EPS)
        nc.vector.reciprocal(nq2[:], nq2[:])
        nc.scalar.activation(cs_[:], cs_[:], AF.Sqrt)      # |cross|
        nc.gpsimd.tensor_tensor(cs_[:], cs_[:], nq2[:], OP.mult)
        curv = tmp.tile([128, H], F32, tag="curv", name="curv")
        nc.vector.tensor_reduce(curv[:], csv, mybir.AxisListType.X, OP.add)
        nc.vector.tensor_scalar_mul(q2[:], q2[:], float(hp["c_q2"]))
        nc.vector.scalar_tensor_tensor(q2[:], curv[:], float(hp["c_curv"]),
                                       q2[:], OP.mult, OP.add)
        nc.vector.tensor_copy(qfv[nt][:, :, 41], q2[:])
        nc.gpsimd.memset(qfv[nt][:, :, 40], 1.0)
        nc.gpsimd.memset(qfv[nt][:, :, 42:64], 0.0)
        for t in range(6):
            pe_transpose(qfT[t][:, nt * 128:(nt + 1) * 128],
                         qf[nt][:, t * 128:(t + 1) * 128], t)

    # ---------------- emission order --------------------------------------
    qpts_pend = {}
    for rt in range(RT):
        emit_ktile(rt)
        if rt == 1:
            for nt in range(NT):
                qpts_pend[nt] = emit_qtile(nt)
                emit_qtransform(nt, qpts_pend[nt])
            for nt in range(NT):
                emit_qstats(nt)

    ppool.close()

    # ---- attention --------------------------------------------------------
    att_ctx = ExitStack()
    apsum = att_ctx.enter_context(tc.tile_pool(name="apsum", bufs=2, space=PS))
    opsum = att_ctx.enter_context(tc.tile_pool(name="opsum", bufs=2, space=PS))
    expT_tiles = [work.tile([128, 2 * N], BF16, name=f"expT{i}")
                  for i in range(4)]
    o_all = [work.tile([128, FEAT * H], BF16, name=f"oall{qt}")
             for qt in range(NT)]
    feats = [work.tile([128, FPAD], BF16, name=f"feats{qt}")
             for qt in range(NT)]
    for qt in range(NT):
        nc.gpsimd.memset(feats[qt][:, FOUT:FOUT + 2], 1.0)
        nc.gpsimd.memset(feats[qt][:, FOUT + 2:], 0.0)
    RUN = 2

    def emit_qk_exp(h):
        t, base = h // 2, (h % 2) * FEAT
        expT = expT_tiles[h % 4]
        for half in range(2):
            aps = apsum.tile([128, 4 * NB], F32, tag="attT", name="aps")
            for kb4 in range(4):
                kb = half * 4 + kb4
                nc.tensor.matmul(
                    aps[:, kb4 * NB:(kb4 + 1) * NB],
                    kfT[t][base:base + FS, kb * 128:(kb + 1) * 128],
                    qfT[t][base:base + FS, :],
                    start=True, stop=True)
            nc.scalar.activation(expT[:, half * 4 * NB:(half + 1) * 4 * NB],
                                 aps[:], AF.Exp, scale=float(sh[h]))

    def emit_av(h):
        expT = expT_tiles[h % 4]
        ot_ps = opsum.tile([VLD, NB], F32, tag="otacc", name="ot_ps")
        for kb in range(8):
            nc.tensor.matmul(
                ot_ps[:],
                va[kb][:, h * VLD:(h + 1) * VLD],
                expT[:, kb * NB:(kb + 1) * NB],
                start=(kb == 0), stop=(kb == 7))
        ot_sb = tmp.tile([VLD, NB], F32R, tag="otsb", name="otsb", bufs=2)
        nc.scalar.copy(ot_sb[:], ot_ps[:])
        for qt in range(NT):
            tp = opsum.tile([128, VLD], F32R, tag="otp", name="tp")
            nc.tensor.transpose(tp[:], ot_sb[:, qt * 128:(qt + 1) * 128],
                                ident_r[:])
            rec = tmp.tile([128, 1], F32, tag="rec", name="rec")
            nc.vector.reciprocal(rec[:], tp[:, 64:65].bitcast(F32))
            nc.vector.tensor_scalar_mul(
                o_all[qt][:, h * FEAT:h * FEAT + 64],
                tp[:, 0:64].bitcast(F32), rec[:])

    GH = 6                   # heads per inverse-transform group

    def emit_inverse(qt, g):
        own = qt
        rot = rt12[:, own * 12:own * 12 + 9]
        tr = rt12[:, own * 12 + 9:own * 12 + 12]
        hs = slice(g * GH, g * GH + GH)
        ovv = o_all[qt][:].rearrange("p (h f) -> p h f", f=FEAT)[:, hs]

        def og(j):  # [128, 6, V] component j of attention-weighted points
            return ovv[:, :, 16 + 8 * j:24 + 8 * j]

        nc.vector.tensor_copy(
            feats[qt][:, g * GH * 16:(g + 1) * GH * 16].rearrange(
                "p (h c) -> p h c", c=16),
            ovv[:, :, 0:16])
        gview = feats[qt][:, 192:FOUT].rearrange(
            "p (h x c) -> p h x c", h=H, c=7)[:, hs]

        ogs = [tmp.tile([128, GH * V], BF16, tag=f"ogs{j}", name=f"ogs{j}")
               for j in range(3)]
        for j in range(3):
            nc.vector.tensor_scalar(
                ogs[j][:].rearrange("p (h x) -> p h x", x=V), og(j),
                tr[:, j:j + 1], None, OP.subtract)
        lc = [tmp.tile([128, GH * V], BF16, tag=f"lc{i}", name=f"lc{i}")
              for i in range(3)]
        ld = [tmp.tile([128, GH * V], BF16, tag=f"ld{i}", name=f"ld{i}")
              for i in range(3)]
        for i in range(3):
            nc.vector.tensor_scalar_mul(lc[i][:], ogs[0][:], rot[:, i:i + 1])
            nc.vector.scalar_tensor_tensor(lc[i][:], ogs[1][:],
                                           rot[:, 3 + i:4 + i],
                                           lc[i][:], OP.mult, OP.add)
            nc.vector.scalar_tensor_tensor(lc[i][:], ogs[2][:],
                                           rot[:, 6 + i:7 + i],
                                           lc[i][:], OP.mult, OP.add)
            ldv = ld[i][:].rearrange("p (h x) -> p h x", x=V)
            nc.vector.tensor_scalar_mul(ldv, og(3), rot[:, i:i + 1])
            nc.vector.scalar_tensor_tensor(ldv, og(4), rot[:, 3 + i:4 + i],
                                           ldv, OP.mult, OP.add)
            nc.vector.scalar_tensor_tensor(ldv, og(5), rot[:, 6 + i:7 + i],
                                           ldv, OP.mult, OP.add)
        n2 = tmp.tile([128, GH * V], F32, tag="n2", name="n2")
        t2b = tmp.tile([128, GH * V], F32, tag="t2b", name="t2b")
        nc.vector.tensor_tensor(n2[:], lc[0][:], lc[0][:], OP.mult)
        for i in (1, 2):
            nc.vector.tensor_tensor(t2b[:], lc[i][:], lc[i][:], OP.mult)
            nc.vector.tensor_tensor(n2[:], n2[:], t2b[:], OP.add)
        nc.scalar.activation(
            gview[:, :, :, 6].rearrange("p h x -> p (h x)"), n2[:], AF.Sqrt)
        for i in range(3):
            nc.vector.tensor_copy(
                gview[:, :, :, i].rearrange("p h x -> p (h x)"), lc[i][:])
        nc.vector.tensor_tensor(n2[:], ld[0][:], ld[0][:], OP.mult)
        for i in (1, 2):
            nc.vector.tensor_tensor(t2b[:], ld[i][:], ld[i][:], OP.mult)
            nc.vector.tensor_tensor(n2[:], n2[:], t2b[:], OP.add)
        nc.scalar.activation(n2[:], n2[:], AF.Sqrt)
        nc.vector.tensor_scalar_max(n2[:], n2[:], EPS)
        nc.vector.reciprocal(n2[:], n2[:])
        rn2 = tmp.tile([128, GH * V], BF16, tag="rn2", name="rn2")
        nc.vector.tensor_copy(rn2[:], n2[:])
        for i in range(3):
            nc.vector.tensor_tensor(
                gview[:, :, :, 3 + i].rearrange("p h x -> p (h x)"),
                ld[i][:], rn2[:], OP.mult)

    for h in range(H + RUN):
        if h < H:
            emit_qk_exp(h)
        if h >= RUN:
            emit_av(h - RUN)
            if h - RUN == 5:
                for qt in range(NT):
                    emit_inverse(qt, 0)
    for qt in range(NT):
        emit_inverse(qt, 1)

    att_ctx.close()

    # ---- output projection (feats -> DRAM -> xbar transpose -> matmul) ---
    opool = ExitStack()
    opsum2 = opool.enter_context(tc.tile_pool(name="opsum2", bufs=2, space=PS))
    fT = [work.tile([128, NB], BF16, name=f"fT{kc}") for kc in range(KCH)]
    for kc in range(KCH):
        for qt in range(NT):
            ps = opsum2.tile([128, 128], BF16, tag="ftps", name="ftps")
            nc.tensor.transpose(ps[:], feats[qt][:, kc * 128:(kc + 1) * 128],
                                identb[:])
            if kc % 2:
                nc.scalar.copy(fT[kc][:, qt * 128:(qt + 1) * 128], ps[:])
            else:
                nc.vector.tensor_copy(fT[kc][:, qt * 128:(qt + 1) * 128],
                                      ps[:])

    for qt in range(NT):
        ps = opsum2.tile([128, CS], F32, tag="oproj")
        for kc in range(KCH):
            nc.tensor.matmul(ps[:], fT[kc][:, qt * 128:(qt + 1) * 128],
                             woutp[:, kc * CS:(kc + 1) * CS],
                             start=(kc == 0), stop=(kc == KCH - 1))
        osb = tmp.tile([128, CS], F32, tag="osb", name="osb")
        nc.scalar.copy(osb[:], ps[:])
        nc.sync.dma_start(out_d[qt * 128:(qt + 1) * 128, :], osb[:])
    opool.close()


def _run(inputs, trace=False):
    hp = _host_prep(inputs)
    nc = _build_program(hp)
    in_maps = []
    for c in range(8):
        m = {
            "sT": hp["sT"][c], "rt12": hp["rt12"][c],
            "wallK": hp["wallK"], "wallQ": hp["wallQ"],
            "woutp": hp["woutp"],
        }
        if hp["has_bias"]:
            m["wbias"] = hp["wbias"]
        in_maps.append(m)
    res = run_bass_kernel_spmd(nc, in_maps, list(range(8)), trace=trace)
    out = np.empty((B, N, CS), np.float32)
    for c in range(8):
        b, qb = c // 4, c % 4
        out[b, qb * NB:(qb + 1) * NB] = res.results[c]["out_loc"]
    return out, res


def kernel(**inputs):
    out, _ = _run(inputs, trace=False)
    return out


def kernel_traced(**inputs):
    return _run(inputs, trace=True)
